# revision 1
# baseline (speedup 1.0000x reference)
# Trainium2 Bass kernel for GQA causal attention (B=2, S=2048, DIM=2048,
# NH=32, NKV=8, HD=64) sharded over 8 NeuronCores: 2-way data parallel over
# batch x 4-way tensor parallel over heads. Each core computes 8 query heads
# (2 KV heads) for one batch element plus a partial wo product; the partial
# sums are reduced on the host (cheap fp32 adds), so no device collective is
# needed.
#
# Self-contained: hardcodes all shapes; only imports the concourse runtime
# available in the environment.
import os
import numpy as np

B, S, DIM = 2, 2048, 2048
NH, NKV, HD = 32, 8, 64
THETA = 10000.0
TPG = 4               # tensor-parallel head-group shards
H_CORE = NH // TPG    # 8 query heads per core
KV_CORE = NKV // TPG  # 2 kv heads per core
SCH = 512             # sequence chunk (matmul moving free dim)
NSCH = S // SCH       # 4
DT = DIM // 128       # 16 contraction tiles for projections
ST = S // 128         # 16 key tiles
N_CORES = 8

# within-head dim permutation: [e0(16) o0(16) e1(16) o1(16)] so that the RoPE
# partner lives 16 partitions away inside each 32-partition quadrant
# (stream_shuffle shuffles within 32-partition quadrants only).
PERM64 = np.array([2 * i for i in range(16)] + [2 * i + 1 for i in range(16)]
                  + [32 + 2 * i for i in range(16)]
                  + [33 + 2 * i for i in range(16)])
HEAD_ORDER_LOCAL = [0, 4, 1, 5, 2, 6, 3, 7]  # (p, p+4) share a 128-row tile
SHUF_MASK = [i ^ 16 for i in range(32)]

_CACHE: dict = {}
LAST_RUN_INFO: dict = {}


def _host_constants():
    freqs = 1.0 / (THETA ** (np.arange(0, HD, 2, dtype=np.float64) / HD))
    ang = np.outer(np.arange(S, dtype=np.float64), freqs)  # [S, 32]
    cosb = np.zeros((128, S), np.float32)
    sinb = np.zeros((128, S), np.float32)
    for row in range(128):
        q, j = divmod(row, 32)
        fi = (q % 2) * 16 + (j % 16)
        cosb[row] = np.cos(ang[:, fi])
        sinb[row] = (-1.0 if j < 16 else 1.0) * np.sin(ang[:, fi])
    kp = np.arange(128)[:, None]
    qf = np.arange(128)[None, :]
    masks = (kp <= qf).astype(np.float32)  # [128, 128] lower triangle
    return cosb, sinb, masks


def _build_program():
    import concourse.bass as bass
    import concourse.mybir as mybir
    import concourse.tile as tile
    from concourse import bacc
    from concourse.masks import make_identity
    from contextlib import ExitStack

    f32 = mybir.dt.float32
    f32r = mybir.dt.float32r
    EXP = mybir.ActivationFunctionType.Exp
    MUL = mybir.AluOpType.mult
    ADD = mybir.AluOpType.add

    nc = bacc.Bacc("TRN2", target_bir_lowering=False, debug=False,
                   enable_asserts=False, num_devices=N_CORES)

    xt_d = nc.dram_tensor("xt", [DIM, S], f32r, kind="ExternalInput").ap()
    wq_d = nc.dram_tensor("wq", [DIM, 512], f32r, kind="ExternalInput").ap()
    wk_d = nc.dram_tensor("wk", [DIM, 128], f32r, kind="ExternalInput").ap()
    wv_d = nc.dram_tensor("wv", [DIM, 128], f32r, kind="ExternalInput").ap()
    wo_d = nc.dram_tensor("wo", [512, DIM], f32r, kind="ExternalInput").ap()
    cos_d = nc.dram_tensor("cosb", [128, S], f32, kind="ExternalInput").ap()
    sin_d = nc.dram_tensor("sinb", [128, S], f32, kind="ExternalInput").ap()
    msk_d = nc.dram_tensor("masks", [128, 128], f32,
                           kind="ExternalInput").ap()
    out_d = nc.dram_tensor("out", [S, DIM], f32, kind="ExternalOutput").ap()

    with tile.TileContext(nc) as tc, ExitStack() as top:
        const = top.enter_context(tc.tile_pool(name="const", bufs=1))
        persist = top.enter_context(tc.tile_pool(name="persist", bufs=1))
        wpool = top.enter_context(tc.tile_pool(name="wpool", bufs=1))
        xpool = top.enter_context(tc.tile_pool(name="xpool", bufs=19))
        rpool = top.enter_context(tc.tile_pool(name="rpool", bufs=1))
        vtpool = top.enter_context(tc.tile_pool(name="vtpool", bufs=1))
        epool = top.enter_context(tc.tile_pool(name="epool", bufs=3))
        rcpool = top.enter_context(tc.tile_pool(name="rcpool", bufs=1))
        wopool = top.enter_context(tc.tile_pool(name="wopool", bufs=7))
        oepool = top.enter_context(tc.tile_pool(name="oepool", bufs=3))
        # one shared PSUM pool, 8 banks via tag aliasing:
        #   q0,q1,q2: QKV accumulators (pass A: qt0/qt1/qt2; pass B:
        #   qt3/kt/vt; V-transpose reuses q0)  sa,sb: score tiles (bcast
        #   reuses sa)  oa,ob: attention accumulators  po: WO accumulator
        psum = top.enter_context(tc.tile_pool(name="psum", bufs=1,
                                              space="PSUM"))

        # ---- weights + x are on the critical path: emit their DMAs first
        wq_sb = wpool.tile([128, DT, 512], f32r, tag="wq")
        wk_sb = wpool.tile([128, DT, 128], f32r, tag="wk")
        wv_sb = wpool.tile([128, DT, 128], f32r, tag="wv")
        wq_r = wq_d.rearrange("(t p) c -> p t c", p=128)
        wk_r = wk_d.rearrange("(t p) c -> p t c", p=128)
        wv_r = wv_d.rearrange("(t p) c -> p t c", p=128)
        for d in range(DT):
            nc.sync.dma_start(wq_sb[:, d, :], wq_r[:, d, :])
        for h in range(4):
            sl = slice(h * DT // 4, (h + 1) * DT // 4)
            nc.sync.dma_start(wk_sb[:, sl, :], wk_r[:, sl, :])
            nc.sync.dma_start(wv_sb[:, sl, :], wv_r[:, sl, :])

        # ---- prefetch chunk 0 of x ahead of the constants: the first
        # matmuls need only wq[:,0,:] and x(0,*) ----
        xts0 = []
        for d in range(DT):
            xt = xpool.tile([128, SCH], f32r, tag="x", name=f"x_0_{d}")
            nc.sync.dma_start(xt[:], xt_d[d * 128:(d + 1) * 128, 0:SCH])
            xts0.append(xt)

        # ---- constants ----
        cos_sb = const.tile([128, S], f32, tag="cos")
        sin_sb = const.tile([128, S], f32, tag="sin")
        msk_sb = const.tile([128, 128], f32, tag="msk")
        nc.sync.dma_start(msk_sb[:], msk_d)
        for h in range(2):
            sl = slice(h * S // 2, (h + 1) * S // 2)
            nc.sync.dma_start(cos_sb[:, sl], cos_d[:, sl])
            nc.sync.dma_start(sin_sb[:, sl], sin_d[:, sl])
        ident = const.tile([128, 128], f32, tag="ident")
        make_identity(nc, ident[:])
        ones_f = const.tile([65, 64], f32, tag="ones_f")
        nc.vector.memset(ones_f[64:65, :], 1.0)
        ones_sb = const.tile([65, 64], f32r, tag="ones")
        nc.scalar.copy(ones_sb[64:65, :], ones_f[64:65, :])
        onecol_f = const.tile([128, 1], f32, tag="onecol_f")
        nc.vector.memset(onecol_f[:], 1.0)

        # ---- persistent activations (attention output overwrites qt) ----
        qt_sb = [persist.tile([128, S], f32r, tag=f"qt{g}", name=f"qt{g}")
                 for g in range(4)]
        kt_sb = persist.tile([128, S], f32r, tag="kt")
        vp_sb = [persist.tile([128, 130], f32r, tag=f"vp{t}", name=f"vp{t}")
                 for t in range(ST)]
        for t in range(ST):
            nc.scalar.copy(vp_sb[t][:, 64:65], onecol_f[:])
            nc.scalar.copy(vp_sb[t][:, 129:130], onecol_f[:])

        def rope_evac(ps, dst, cosc, sinc, nm):
            t1 = rpool.tile([128, SCH], f32, tag="r1", name=f"r1_{nm}")
            nc.vector.stream_shuffle(t1[:], ps[:], mask=SHUF_MASK)
            nc.vector.tensor_tensor(dst, ps[:], cosc, MUL)
            t2 = rpool.tile([128, SCH], f32, tag="r2", name=f"r2_{nm}")
            nc.vector.tensor_tensor(t2[:], t1[:], sinc, MUL)
            nc.vector.tensor_tensor(dst, dst, t2[:], ADD)

        for c in range(NSCH):
            cs = slice(c * SCH, (c + 1) * SCH)
            cosc, sinc = cos_sb[:, cs], sin_sb[:, cs]
            # ---- x tiles for this chunk (chunk 0 was prefetched) ----
            if c == 0:
                xts = xts0
            else:
                xts = []
                for d in range(DT):
                    xt = xpool.tile([128, SCH], f32r, tag="x",
                                    name=f"x_{c}_{d}")
                    nc.sync.dma_start(xt[:], xt_d[d * 128:(d + 1) * 128, cs])
                    xts.append(xt)
            # three 2-output passes over the resident x tiles (2 psum tags)
            for g in range(2):
                ps0 = psum.tile([128, SCH], f32, tag="q0",
                                name=f"psq{2*g}_{c}")
                ps1 = psum.tile([128, SCH], f32, tag="q1",
                                name=f"psq{2*g+1}_{c}")
                for d in range(DT):
                    st, sp = (d == 0), (d == DT - 1)
                    nc.tensor.matmul(
                        ps0[:], wq_sb[:, d, 2 * g * 128:(2 * g + 1) * 128],
                        xts[d][:], start=st, stop=sp)
                    nc.tensor.matmul(
                        ps1[:],
                        wq_sb[:, d, (2 * g + 1) * 128:(2 * g + 2) * 128],
                        xts[d][:], start=st, stop=sp)
                rope_evac(ps0, qt_sb[2 * g][:, cs], cosc, sinc,
                          f"a{c}_{2*g}")
                rope_evac(ps1, qt_sb[2 * g + 1][:, cs], cosc, sinc,
                          f"a{c}_{2*g+1}")
            psk = psum.tile([128, SCH], f32, tag="q0", name=f"psk_{c}")
            psv = psum.tile([128, SCH], f32, tag="q1", name=f"psv_{c}")
            for d in range(DT):
                st, sp = (d == 0), (d == DT - 1)
                nc.tensor.matmul(psk[:], wk_sb[:, d, :], xts[d][:],
                                 start=st, stop=sp)
                nc.tensor.matmul(psv[:], wv_sb[:, d, :], xts[d][:],
                                 start=st, stop=sp)
            rope_evac(psk, kt_sb[:, cs], cosc, sinc, f"k{c}")
            vt = vtpool.tile([128, SCH], f32, tag="vt", name=f"vt_{c}")
            nc.scalar.copy(vt[:], psv[:])
            for rr in range(4):
                kt_i = 4 * c + rr
                pst = psum.tile([128, 128], f32, tag="q0",
                                name=f"pst_{c}_{rr}")
                nc.tensor.transpose(pst[:], vt[:, rr * 128:(rr + 1) * 128],
                                    ident[:])
                nc.scalar.copy(vp_sb[kt_i][:, 0:64], pst[:, 0:64])
                nc.scalar.copy(vp_sb[kt_i][:, 65:129], pst[:, 64:128])

            # ---- attention for this chunk ----
            nkt = 4 * (c + 1)
            for g in range(4):
                pa = psum.tile([65, SCH], f32, tag="oa", name=f"oa_{c}_{g}")
                pb = psum.tile([65, SCH], f32, tag="ob", name=f"ob_{c}_{g}")
                for t in range(nkt):
                    ks = slice(t * 128, (t + 1) * 128)
                    rr = t - 4 * c
                    lo = max(rr, 0) * 128  # causally-live columns start here
                    qs = slice(c * SCH + lo, (c + 1) * SCH)
                    sa = psum.tile([128, SCH], f32, tag="sa", bufs=2,
                                   name=f"sa_{c}_{g}_{t}")
                    sb_ = psum.tile([128, SCH], f32, tag="sb", bufs=2,
                                    name=f"sb_{c}_{g}_{t}")
                    nc.tensor.matmul(sa[:, lo:], kt_sb[0:64, ks],
                                     qt_sb[g][0:64, qs],
                                     start=True, stop=True)
                    nc.tensor.matmul(sb_[:, lo:], kt_sb[64:128, ks],
                                     qt_sb[g][64:128, qs],
                                     start=True, stop=True)
                    ea = epool.tile([128, SCH], f32r, tag="ea",
                                    name=f"ea_{c}_{g}_{t}")
                    eb = epool.tile([128, SCH], f32r, tag="eb",
                                    name=f"eb_{c}_{g}_{t}")
                    nc.scalar.activation(ea[:, lo:], sa[:, lo:], EXP,
                                         scale=0.125)
                    nc.scalar.activation(eb[:, lo:], sb_[:, lo:], EXP,
                                         scale=0.125)
                    if rr >= 0:  # mask the mixed 128-column block
                        mb = slice(lo, lo + 128)
                        nc.vector.tensor_tensor(ea[:, mb], ea[:, mb],
                                                msk_sb[:], MUL)
                        nc.vector.tensor_tensor(eb[:, mb], eb[:, mb],
                                                msk_sb[:], MUL)
                    st, sp = (t == 0), (t == nkt - 1)
                    nc.tensor.matmul(pa[:, lo:], vp_sb[t][:, 0:65],
                                     ea[:, lo:], start=st, stop=sp)
                    nc.tensor.matmul(pb[:, lo:], vp_sb[t][:, 65:130],
                                     eb[:, lo:], start=st, stop=sp)
                # normalize rows 0:64 by row 64; write back into qt (the
                # chunk-c columns of qt are dead once the scores are done)
                for half, ps in ((0, pa), (1, pb)):
                    acc = rcpool.tile([65, SCH], f32, tag=f"acc{half}",
                                      name=f"acc{half}_{c}_{g}")
                    nc.vector.tensor_copy(acc[:], ps[:])
                    rc = rcpool.tile([65, SCH], f32, tag=f"rc{half}",
                                     name=f"rc{half}_{c}_{g}")
                    nc.vector.reciprocal(rc[64:65, :], acc[64:65, :])
                    rcr = rcpool.tile([65, SCH], f32r, tag=f"rcr{half}",
                                      name=f"rcr{half}_{c}_{g}")
                    nc.vector.tensor_copy(rcr[64:65, :], rc[64:65, :])
                    bc = psum.tile([64, SCH], f32, tag="sa", bufs=2,
                                   name=f"bc{half}_{c}_{g}")
                    nc.tensor.matmul(bc[:], ones_sb[64:65, :], rcr[64:65, :],
                                     start=True, stop=True)
                    bcs = rcpool.tile([64, SCH], f32, tag=f"bcs{half}",
                                      name=f"bcs{half}_{c}_{g}")
                    nc.vector.tensor_copy(bcs[:], bc[:])
                    dst = qt_sb[g][half * 64:(half + 1) * 64, cs]
                    nc.vector.tensor_tensor(dst, acc[0:64, :], bcs[:], MUL)

            # ---- output projection for this chunk ----
            for e in range(4):
                wo_e = []
                for g in range(4):
                    w = wopool.tile([128, 512], f32r, tag="wo",
                                    name=f"wo_{c}_{e}_{g}")
                    nc.sync.dma_start(
                        w[:], wo_d[g * 128:(g + 1) * 128,
                                   e * 512:(e + 1) * 512])
                    wo_e.append(w)
                for m in range(4 * c, 4 * c + 4):
                    ms = slice(m * 128, (m + 1) * 128)
                    po = psum.tile([128, 512], f32,
                                   tag="oa" if (m + e) % 2 == 0 else "ob",
                                   name=f"po_{m}_{e}")
                    for g in range(4):
                        nc.tensor.matmul(po[:], qt_sb[g][:, ms],
                                         wo_e[g][:],
                                         start=(g == 0), stop=(g == 3))
                    ot = oepool.tile([128, 512], f32, tag="ot",
                                     name=f"ot_{m}_{e}")
                    nc.vector.tensor_copy(ot[:], po[:])
                    nc.sync.dma_start(out_d[ms, e * 512:(e + 1) * 512],
                                      ot[:])

    nc.compile()
    return nc


def get_program():
    if "nc" not in _CACHE:
        _CACHE["nc"] = _build_program()
    return _CACHE["nc"]


def shard_inputs(x, wq, wk, wv, wo):
    """Returns in_maps for cores 0..7; core = b*4 + g."""
    cosb, sinb, masks = _host_constants()
    in_maps = []
    for b in range(B):
        xT = np.ascontiguousarray(np.asarray(x[b], np.float32).T)
        for g in range(TPG):
            qheads = [H_CORE * g + h for h in HEAD_ORDER_LOCAL]
            qcols = np.concatenate([h * HD + PERM64 for h in qheads])
            kvheads = [KV_CORE * g, KV_CORE * g + 1]
            kcols = np.concatenate([h * HD + PERM64 for h in kvheads])
            vcols = np.concatenate([h * HD + np.arange(HD) for h in kvheads])
            worows = np.concatenate([h * HD + np.arange(HD) for h in qheads])
            in_maps.append({
                "xt": xT,
                "wq": np.ascontiguousarray(np.asarray(wq, np.float32)[:, qcols]),
                "wk": np.ascontiguousarray(np.asarray(wk, np.float32)[:, kcols]),
                "wv": np.ascontiguousarray(np.asarray(wv, np.float32)[:, vcols]),
                "wo": np.ascontiguousarray(np.asarray(wo, np.float32)[worows, :]),
                "cosb": cosb,
                "sinb": sinb,
                "masks": masks,
            })
    return in_maps


def _install_trace_shim():
    """Dev-only: synthesize the antenv.axon_hooks NTFF profile hook (this
    image's antenv lacks it) so trace=True works under axon. Safe no-op on
    any failure."""
    import sys
    import types
    try:
        import antenv
        if getattr(antenv, "axon_hooks", None) is not None:
            return
        from trn_agent_boot.trn_boot import _ntff_profile_via_ctypes
        hook = _ntff_profile_via_ctypes("/opt/axon/libaxon_pjrt.so")
        mod = types.ModuleType("antenv.axon_hooks")
        mod.get_axon_ntff_profile_hook = lambda: hook
        mod.set_axon_ntff_profile_hook = lambda h: None
        sys.modules["antenv.axon_hooks"] = mod
        antenv.axon_hooks = mod
        from concourse import bass_utils
        bass_utils.upload_artifacts = lambda tmpdir: "local://unuploaded"
    except Exception as e:  # pragma: no cover
        print(f"trace shim unavailable: {e}")


def kernel(x, wq, wk, wv, wo):
    from concourse import bass_utils

    nc = get_program()
    in_maps = shard_inputs(x, wq, wk, wv, wo)
    trace = os.environ.get("KERNEL_TRACE", "0") == "1"
    if trace:
        _install_trace_shim()
    res = bass_utils.run_bass_kernel_spmd(
        nc, in_maps, core_ids=list(range(N_CORES)), trace=trace)
    LAST_RUN_INFO.clear()
    LAST_RUN_INFO.update(
        exec_time_ns=res.exec_time_ns,
        mean_exec_time_ns=res.mean_exec_time_ns,
        trace=(res.instructions_and_trace[1]
               if res.instructions_and_trace else None),
    )
    out = np.zeros((B, S, DIM), np.float32)
    for b in range(B):
        for g in range(TPG):
            out[b] += res.results[b * TPG + g]["out"]
    return out


def time_device_exec(inputs, iters=6):
    """Test-only: time warm PJRT executes with device-resident inputs.
    Returns per-iteration wall seconds (upper bound on device exec)."""
    import jax
    import concourse.mybir as mybir
    from jax.sharding import Mesh, PartitionSpec
    from jax.experimental.shard_map import shard_map
    from concourse.bass2jax import (_bass_exec_p, partition_id_tensor,
                                    install_neuronx_cc_hook)
    import time as _time

    install_neuronx_cc_hook()
    nc = get_program()
    in_maps = shard_inputs(**inputs) if isinstance(inputs, dict) else inputs

    partition_name = (nc.partition_id_tensor.name
                      if nc.partition_id_tensor else None)
    in_names, out_names, out_avals, zero_outs = [], [], [], []
    for alloc in nc.m.functions[0].allocations:
        if not isinstance(alloc, mybir.MemoryLocationSet):
            continue
        name = alloc.memorylocations[0].name
        if alloc.kind == "ExternalInput":
            if name != partition_name:
                in_names.append(name)
        elif alloc.kind == "ExternalOutput":
            shape = tuple(alloc.tensor_shape)
            dtype = mybir.dt.np(alloc.dtype)
            out_names.append(name)
            out_avals.append(jax.core.ShapedArray(shape, dtype))
            zero_outs.append(np.zeros(shape, dtype))
    n_params = len(in_names)
    n_outs = len(out_avals)
    all_in_names = list(in_names) + list(out_names)
    if partition_name is not None:
        all_in_names.append(partition_name)
    donate = tuple(range(n_params, n_params + n_outs))

    def _body(*args):
        operands = list(args)
        if partition_name is not None:
            operands.append(partition_id_tensor())
        outs = _bass_exec_p.bind(
            *operands, out_avals=tuple(out_avals),
            in_names=tuple(all_in_names), out_names=tuple(out_names),
            lowering_input_output_aliases=(), sim_require_finite=True,
            sim_require_nnan=True, nc=nc)
        return tuple(outs)

    devices = jax.devices()[:N_CORES]
    mesh = Mesh(np.asarray(devices), ("core",))
    sharded = jax.jit(
        shard_map(_body, mesh=mesh,
                  in_specs=(PartitionSpec("core"),) * (n_params + n_outs),
                  out_specs=(PartitionSpec("core"),) * n_outs,
                  check_rep=False),
        donate_argnums=donate, keep_unused=True)

    sh = jax.sharding.NamedSharding(mesh, PartitionSpec("core"))
    concat_in = [np.concatenate([np.asarray(in_maps[c][nm])
                                 for c in range(N_CORES)], axis=0)
                 for nm in in_names]
    in_dev = [jax.device_put(a, sh) for a in concat_in]
    for a in in_dev:
        a.block_until_ready()
    times = []
    for _ in range(iters):
        zs = [jax.device_put(np.zeros((N_CORES * z.shape[0], *z.shape[1:]),
                                      z.dtype), sh) for z in zero_outs]
        for z in zs:
            z.block_until_ready()
        t0 = _time.time()
        outs = sharded(*in_dev, *zs)
        for o in outs:
            o.block_until_ready()
        times.append(_time.time() - t0)
    return times

